# revision 33
# baseline (speedup 1.0000x reference)
"""Trainium2 Bass kernel for nn_EventEmulator (DVS event-camera emulator).

Reference computation per pixel (B*H*W independent pixels, F=16 frames):
  l_f = lin_log(frame_f)                       (linear below 20, log above)
  base_0 = l_0
  per step f=1..15:
    d = l_f - base
    p = floor(relu(d)/pt);  n = floor(relu(-d)/nt)
    base += p*pt - n*nt
    c_f = p - n
  voxel_k = sum_f w[f,k] * c_f                 (bilinear temporal weights, 5 bins)

Strategy:
 - Shard H=512 across 8 cores (64 rows each) -> 131072 pixels/core laid out
   as [128 partitions x 1024] fp32 tiles (2 column-chunks of 512 so the
   sequential per-pixel scan pipelines across engines).
 - Per step (matching the reference's arithmetic order):
     d  = l_f - base                              [DVE tt]
     p  = floor(relu(d * (1/pt)))                 [custom DVE op, fp16 out]
     nn = -floor(relu(d * (-1/nt)))               [custom DVE op, fp16 out]
     m1 = p*pt ; m2 = nn*nt                       [DVE tt, fp16 x fp32]
     base = (base + m1) + m2                      [GPSIMD tt]
 - floor via fp32 magic-number round ((x - 0.5 + 1.5*2^23) - 1.5*2^23)
   fused with the relu+scale inside one custom DVE instruction each.
 - lin_log via the exact identity  min(max(ln(x+1e-9), 0.2), x*ln(20)/20)
   (one ACT Ln pass + one custom DVE combine).
 - Voxel accumulation on the TensorEngine: psum_bin += (w*I).T @ p (+ nn)
   with fp16 diagonal weight matrices; p/nn are small integers, exact in
   fp16. Finished bins: PSUM -> SBUF (ACT copy) -> DRAM.
 - Engines: ACT ln + psum copies, DVE custom ops + mixed-dtype tt, GPSIMD
   (Pool) the base-update adds, PE voxel matmuls, HWDGE DMAs.
 - TimelineSim-predicted single-core exec: ~130 us.
"""

import os
import sys

for _p in ("/opt/trn_rl_repo", "/root/.axon_site/_ro/trn_rl_repo"):
    if os.path.isdir(_p):
        sys.path.insert(0, _p)
        break

import numpy as np

import concourse.bacc as bacc
import concourse.mybir as mybir
import concourse.tile as tile
from concourse import bass_utils
from concourse.dve_ops import (
    DveOp,
    OPS,
    CUSTOM_DVE_SPECS,
    _SUB_OPCODE_FOR_NAME,
    _CUSTOM_DVE_ROW_BASE,
)
from concourse.dve_spec import Spec, Src0, Src1, C0, C1, C2, Zero, relu, maxx, minn, lower
from concourse.dve_uop import DveOpSpec

# ---------------------------------------------------------------- constants
B, F, H, W = 4, 16, 512, 512
N_CORES = 8
H_SH = H // N_CORES          # 64 rows per core
P = 128                      # partitions
NPIX = B * H_SH * W          # 131072 pixels per core
NCOL = NPIX // P             # 1024
NCHUNK = 2
CW = NCOL // NCHUNK          # 512 columns per chunk
HH = H_SH // 2               # 32 partition-rows per batch

NUM_BINS = 5
MAGIC = 12582912.0           # 1.5 * 2^23: fp32 round-to-nearest-int magic
HALF = 0.5
F_LIN = float(np.float32(np.log(np.float32(20.0)) / np.float32(20.0)))
LN_BIAS = 1e-9
LN_CLAMP = 0.2               # any value in [0.1792, ln(20)] works

FP32 = mybir.dt.float32
FP16 = mybir.dt.float16


# ------------------------------------------------------- custom DVE ops
def _register_op(name, spec):
    """Register a custom DVE op at runtime (self-computed uops sha)."""
    for existing in OPS:
        if existing.name == name:
            return existing
    shas = {}
    for ver in ("v3", "v4"):
        s = DveOpSpec(name=name, opcode=0, uops=lower(spec, ver=ver), rd1_en=True)
        shas[ver] = s.sha(ver)
    op = DveOp(name, spec, subdim=False, uops_sha=shas)
    OPS.append(op)
    CUSTOM_DVE_SPECS[name] = spec
    _SUB_OPCODE_FOR_NAME[name] = _CUSTOM_DVE_ROW_BASE + len(OPS) - 1
    return op


def _FF(x):
    # floor(x) for x >= 0 (ties-at-integers round half-to-even; measure zero)
    return ((x - C2) + C1) - C1


def _np_ff(x):
    return (np.round((x - 0.5).astype(np.float32) + MAGIC) - MAGIC).astype(np.float32)


# p = floor(relu(d * ipt));  s1=MAGIC, imm2=0.5
EVT_PF = _register_op(
    "EVT_PF2",
    Spec(
        body=_FF(relu(Src0 * Src1)),
        reference=lambda in0, in1, s0, s1, imm2: _np_ff(
            np.maximum(in0 * in1, 0.0).astype(np.float32)
        ),
    ),
)

# nn = -floor(relu(d * nint))  (nint = -1/nt);  s1=MAGIC, imm2=0.5
EVT_NF = _register_op(
    "EVT_NF2",
    Spec(
        body=Zero - _FF(relu(Src0 * Src1)),
        reference=lambda in0, in1, s0, s1, imm2: -_np_ff(
            np.maximum(in0 * in1, 0.0).astype(np.float32)
        ),
    ),
)

# l = min(max(ln_x, s0), x * s1)   (lin_log combine)
EVT_LC = _register_op(
    "EVT_LC",
    Spec(
        body=minn(maxx(Src0, C0), Src1 * C1),
        reference=lambda in0, in1, s0, s1, imm2: np.minimum(
            np.maximum(in0, s0), in1 * s1
        ).astype(np.float32),
    ),
)


# ------------------------------------------------------- temporal weights
def _weight_table():
    """Per frame f=1..15: list of (slot, bin, w). slot indexes the diag tensor."""
    t = np.linspace(np.float32(0.0), np.float32(NUM_BINS - 1), F, dtype=np.float32)[1:]
    bins = np.arange(NUM_BINS, dtype=np.float32)
    wts = np.maximum(0.0, 1.0 - np.abs(t[:, None] - bins[None, :])).astype(np.float32)
    table = []  # [(f, [(slot,k,w), ...])]
    slots = []  # w value per slot
    for fi in range(15):
        touches = []
        for k in range(NUM_BINS):
            w = float(wts[fi, k])
            if w > 0.0:
                touches.append((len(slots), k, w))
                slots.append(w)
        table.append(touches)
    return table, slots


W_TABLE, W_SLOTS = _weight_table()
N_SLOTS = len(W_SLOTS)  # 29

# last frame index (0-based step index into W_TABLE) touching each bin
BIN_LAST = {}
for fi, touches in enumerate(W_TABLE):
    for _, k, _ in touches:
        BIN_LAST[k] = fi


def _diag_host():
    d = np.zeros((P, N_SLOTS * P), dtype=np.float16)
    eye = np.eye(P, dtype=np.float16)
    for j, w in enumerate(W_SLOTS):
        d[:, j * P : (j + 1) * P] = (np.float16(w) * eye).astype(np.float16)
    return d


# ------------------------------------------------------------ build kernel
_CACHED_NC = None


def _build_nc():
    nc = bacc.Bacc(
        "TRN2",
        target_bir_lowering=False,
        debug=False,
        enable_asserts=False,
        num_devices=N_CORES,
    )

    frames = nc.dram_tensor("frames", [B, F, H_SH, W], FP32, kind="ExternalInput").ap()
    pos = nc.dram_tensor("pos", [B, 1, H_SH, W], FP32, kind="ExternalInput").ap()
    neg = nc.dram_tensor("neg", [B, 1, H_SH, W], FP32, kind="ExternalInput").ap()
    diag = nc.dram_tensor("diag", [P, N_SLOTS * P], FP16, kind="ExternalInput").ap()
    vox = nc.dram_tensor("vox", [B, NUM_BINS, H_SH, W], FP32, kind="ExternalOutput").ap()

    ALU = mybir.AluOpType

    with tile.TileContext(nc) as tc:
        with (
            tc.tile_pool(name="const", bufs=1) as cpool,
            tc.tile_pool(name="frames", bufs=4) as xpool,
            tc.tile_pool(name="lp", bufs=4) as lppool,
            tc.tile_pool(name="state", bufs=4) as spool,
            tc.tile_pool(name="tmp", bufs=4) as tpool,
            tc.tile_pool(name="stage", bufs=2) as stpool,
            tc.tile_pool(name="vox", bufs=2, space="PSUM") as vpool,
        ):
            # ---------------- constants / thresholds prep
            dg = cpool.tile([P, N_SLOTS * P], FP16, tag="diag")
            nc.sync.dma_start(dg[:], diag[:])

            lnb = cpool.tile([P, 1], FP32, tag="lnb")
            nc.vector.memset(lnb[:], LN_BIAS)

            pt = cpool.tile([P, NCOL], FP32, tag="pt")
            nt = cpool.tile([P, NCOL], FP32, tag="nt")
            for b in range(B):
                nc.sync.dma_start(
                    pt[b * HH : (b + 1) * HH, :],
                    pos[b, 0].rearrange("(hh h) w -> hh (h w)", h=2),
                )
                nc.sync.dma_start(
                    nt[b * HH : (b + 1) * HH, :],
                    neg[b, 0].rearrange("(hh h) w -> hh (h w)", h=2),
                )
            ipt = cpool.tile([P, NCOL], FP32, tag="ipt")
            int_ = cpool.tile([P, NCOL], FP32, tag="int")
            rscr = cpool.tile([P, NCOL], FP32, tag="rscr")
            nc.vector.reciprocal_approx_accurate(ipt[:], pt[:], rscr[:])
            nc.vector.reciprocal_approx_accurate(int_[:], nt[:], rscr[:])
            nint = cpool.tile([P, NCOL], FP32, tag="nint")
            nc.vector.tensor_scalar_mul(nint[:], int_[:], -1.0)

            # ---------------- frame prep helper
            lp_tiles = {}

            def prep_frame(f):
                x = xpool.tile([P, NCOL], FP32, tag="x", bufs=3)
                for b in range(B):
                    nc.sync.dma_start(
                        x[b * HH : (b + 1) * HH, :],
                        frames[b, f].rearrange("(hh h) w -> hh (h w)", h=2),
                    )
                lnx = xpool.tile([P, NCOL], FP32, tag="lnx", bufs=2)
                nc.scalar.activation(
                    lnx[:], x[:], mybir.ActivationFunctionType.Ln, bias=lnb[:], scale=1.0
                )
                l = lppool.tile([P, NCOL], FP32, tag="l", name=f"ltile{f}")
                nc.vector._custom_dve(
                    EVT_LC, out=l[:], in0=lnx[:], in1=x[:], s0=LN_CLAMP, s1=F_LIN
                )
                lp_tiles[f] = l

            # ---------------- psum bins
            vox_psum = {}     # (k, c) -> psum tile
            mm_count = {}     # (k, c) -> matmuls emitted so far
            MM_TOTAL = {}     # (k, c) -> total matmuls that will be emitted
            for k in range(NUM_BINS):
                n_frames = sum(
                    1 for touches in W_TABLE for (_, kk, _) in touches if kk == k
                )
                for c in range(NCHUNK):
                    MM_TOTAL[(k, c)] = 2 * n_frames

            def bin_matmul(k, c, w_slot, rhs_ap):
                key = (k, c)
                if key not in vox_psum:
                    vox_psum[key] = vpool.tile([P, CW], FP32, tag=f"vox{c}", name=f"voxp{k}_{c}")
                    mm_count[key] = 0
                first = mm_count[key] == 0
                last = mm_count[key] == MM_TOTAL[key] - 1
                nc.tensor.matmul(
                    vox_psum[key][:],
                    dg[:, w_slot * P : (w_slot + 1) * P],
                    rhs_ap,
                    start=first,
                    stop=last,
                )
                mm_count[key] += 1

            def bin_flush(k):
                # psum -> sbuf -> dram
                for c in range(NCHUNK):
                    st = stpool.tile([P, CW], FP32, tag=f"stage{c}")
                    nc.scalar.copy(st[:], vox_psum[(k, c)][:])
                    ch = (c * CW) // W
                    cw0 = (c * CW) % W
                    for b in range(B):
                        nc.sync.dma_start(
                            vox[b, k].rearrange("(hh h) w -> h hh w", h=2)[ch][
                                :, cw0 : cw0 + CW
                            ],
                            st[b * HH : (b + 1) * HH, :],
                        )

            # ---------------- frame 0 + state init
            prep_frame(0)
            bp = {}
            for c in range(NCHUNK):
                cols = slice(c * CW, (c + 1) * CW)
                t0 = spool.tile([P, CW], FP32, tag=f"bp{c}", name=f"bpinit{c}")
                nc.vector.tensor_copy(t0[:], lp_tiles[0][:, cols])
                bp[c] = t0

            # ---------------- scan
            for fi in range(15):
                f = fi + 1
                prep_frame(f)
                lp = lp_tiles[f]
                touches = W_TABLE[fi]

                for c in range(NCHUNK):
                    cols = slice(c * CW, (c + 1) * CW)
                    lpc = lp[:, cols]

                    d = tpool.tile([P, CW], FP32, tag=f"d{c}")
                    nc.vector.tensor_tensor(d[:], lpc, bp[c][:], ALU.subtract)
                    p16 = tpool.tile([P, CW], FP16, tag=f"p{c}")
                    nc.vector._custom_dve(
                        EVT_PF, out=p16[:], in0=d[:], in1=ipt[:, cols], s1=MAGIC, imm2=HALF
                    )
                    nn16 = tpool.tile([P, CW], FP16, tag=f"n{c}")
                    nc.vector._custom_dve(
                        EVT_NF, out=nn16[:], in0=d[:], in1=nint[:, cols], s1=MAGIC, imm2=HALF
                    )
                    m1 = tpool.tile([P, CW], FP32, tag=f"m1{c}")
                    nc.vector.tensor_tensor(m1[:], p16[:], pt[:, cols], ALU.mult)
                    m2 = tpool.tile([P, CW], FP32, tag=f"m2{c}")
                    nc.vector.tensor_tensor(m2[:], nn16[:], nt[:, cols], ALU.mult)
                    b1 = spool.tile([P, CW], FP32, tag=f"bh{c}")
                    nc.gpsimd.tensor_tensor(b1[:], bp[c][:], m1[:], ALU.add)
                    bpn = spool.tile([P, CW], FP32, tag=f"bp{c}")
                    nc.gpsimd.tensor_tensor(bpn[:], b1[:], m2[:], ALU.add)
                    bp[c] = bpn

                    for slot, k, _w in touches:
                        bin_matmul(k, c, slot, p16[:])
                        bin_matmul(k, c, slot, nn16[:])

                # flush any bin whose last frame was fi
                for k in range(NUM_BINS):
                    if BIN_LAST[k] == fi:
                        bin_flush(k)

    nc.compile()
    return nc


def _get_nc():
    global _CACHED_NC
    if _CACHED_NC is None:
        _CACHED_NC = _build_nc()
    return _CACHED_NC


# ------------------------------------------------------------------ driver
def kernel(frames, t_frames, pos_thres, neg_thres):
    frames = np.asarray(frames, dtype=np.float32)
    pos_thres = np.asarray(pos_thres, dtype=np.float32)
    neg_thres = np.asarray(neg_thres, dtype=np.float32)

    nc = _get_nc()
    dg = _diag_host()
    in_maps = []
    for c in range(N_CORES):
        hs = slice(c * H_SH, (c + 1) * H_SH)
        in_maps.append(
            {
                "frames": np.ascontiguousarray(frames[:, :, hs, :]),
                "pos": np.ascontiguousarray(pos_thres[:, :, hs, :]),
                "neg": np.ascontiguousarray(neg_thres[:, :, hs, :]),
                "diag": dg,
            }
        )

    res = bass_utils.run_bass_kernel_spmd(nc, in_maps, core_ids=list(range(N_CORES)))
    out = np.empty((B, NUM_BINS, H, W), dtype=np.float32)
    for c in range(N_CORES):
        hs = slice(c * H_SH, (c + 1) * H_SH)
        out[:, :, hs, :] = res.results[c]["vox"]
    return out
